# revision 5
# baseline (speedup 1.0000x reference)
"""MoE layer (16 experts, top-4, silu-gated FFN + shared expert) on 8 trn2 cores.

Strategy (expert-parallel, host-side dispatch):
  - Host computes the router (softmax + top-4 + renormalize) in numpy —
    0.2% of total FLOPs — and gathers each expert's tokens into a padded
    [capacity] batch (classic MoE dispatch, done host-side instead of
    device all-to-all).
  - Each of the 8 cores holds 2 experts (weights resident in SBUF, bf16)
    and runs the dense silu-gated FFN over its experts' gathered tokens,
    scaling activations by the combine weights before the down-projection
    so partial outputs can be scatter-added on the host.
  - The shared expert is data-parallel: core i handles tokens
    [i*256, (i+1)*256).
  - All activations/weights are bf16 (PE: 1 cycle/row vs 2 for fp32),
    accumulation in fp32 PSUM.

Device layout: activations kept transposed ([d_model, tokens]: feature on
partitions, tokens on the free dim) so both matmuls feed the PE without any
on-device transpose; combine weights arrive pre-broadcast as [128, C] rows.
DMA is batched (one dma_start per tensor) except expert 0's first-needed
tensors, which are split in k-halves so the PE can start ~4us in. Token
chunks are equal halves (e.g. 288+288 for C=576) so no chunk is so short
that LDWEIGHTS dominates.
"""

import os
import numpy as np
import ml_dtypes

DIM = 1024
HID = 512
E = 16
TOPK = 4
NCORES = 8
EPC = E // NCORES  # experts per core
T = 2048
S = T // NCORES  # shared-expert tokens per core

BF16 = ml_dtypes.bfloat16
OUT_BF16 = os.environ.get("KERNEL_OUT_F32", "0") != "1"

_CACHE = {}


def _chunks(total):
    if total <= 512:
        return [(0, total)]
    nch = -(-total // 512)
    base = -(-total // (nch * 16)) * 16
    out, n0 = [], 0
    while n0 < total:
        n = min(base, total - n0)
        out.append((n0, n))
        n0 += n
    return out


def _build(C: int):
    """Build + schedule the SPMD Tile kernel for per-expert capacity C."""
    import concourse.tile as tile
    import concourse.mybir as mybir
    from concourse import bacc

    f32 = mybir.dt.float32
    bf16 = mybir.dt.bfloat16
    fout = bf16 if OUT_BF16 else f32

    nc = bacc.Bacc("TRN2", target_bir_lowering=False, debug=False,
                   num_devices=NCORES)

    xe = nc.dram_tensor("xe", [EPC, DIM, C], bf16, kind="ExternalInput")
    cb = nc.dram_tensor("cb", [EPC, 128, C], f32, kind="ExternalInput")
    we1 = nc.dram_tensor("we1", [EPC, DIM, HID], bf16, kind="ExternalInput")
    we3 = nc.dram_tensor("we3", [EPC, DIM, HID], bf16, kind="ExternalInput")
    we2 = nc.dram_tensor("we2", [EPC, HID, DIM], bf16, kind="ExternalInput")
    xs = nc.dram_tensor("xs", [DIM, S], bf16, kind="ExternalInput")
    ws1 = nc.dram_tensor("ws1", [DIM, HID], bf16, kind="ExternalInput")
    ws3 = nc.dram_tensor("ws3", [DIM, HID], bf16, kind="ExternalInput")
    ws2 = nc.dram_tensor("ws2", [HID, DIM], bf16, kind="ExternalInput")
    oute = nc.dram_tensor("oute", [EPC, DIM, C], fout, kind="ExternalOutput")
    outs = nc.dram_tensor("outs", [DIM, S], fout, kind="ExternalOutput")

    DK = DIM // 128   # 8 contraction tiles for the up-projections
    HK = HID // 128   # 4 contraction tiles for the down-projection
    KH = DK // 2

    def as_pkf(ap):
        return ap.rearrange("(k p) f -> p k f", p=128)

    with tile.TileContext(nc) as tc:
        with (
            tc.tile_pool(name="wts", bufs=1) as wts,
            tc.tile_pool(name="acts", bufs=1) as actp,
            tc.tile_pool(name="work", bufs=2) as work,
            tc.tile_pool(name="ost", bufs=2) as ostp,
            tc.tile_pool(name="ph", bufs=2, space="PSUM") as ph,
            tc.tile_pool(name="po", bufs=2, space="PSUM") as po,
        ):
            jobs = []
            # expert 0: first-needed tensors in k-halves for a fast start
            w1h = [wts.tile([128, KH, HID], bf16, name=f"w1_0{h}")
                   for h in range(2)]
            w3h = [wts.tile([128, KH, HID], bf16, name=f"w3_0{h}")
                   for h in range(2)]
            xeh = [actp.tile([128, KH, C], bf16, name=f"xe_0{h}")
                   for h in range(2)]
            cb0 = actp.tile([128, C], f32, name="cbt_0")
            w20 = wts.tile([128, HK, DIM], bf16, name="w2_0")
            nc.sync.dma_start(out=w1h[0][:],
                              in_=as_pkf(we1[0])[:, 0:KH, :])
            nc.scalar.dma_start(out=xeh[0][:],
                                in_=as_pkf(xe[0])[:, 0:KH, :])
            nc.sync.dma_start(out=w1h[1][:],
                              in_=as_pkf(we1[0])[:, KH:DK, :])
            nc.scalar.dma_start(out=xeh[1][:],
                                in_=as_pkf(xe[0])[:, KH:DK, :])
            nc.sync.dma_start(out=w3h[0][:],
                              in_=as_pkf(we3[0])[:, 0:KH, :])
            nc.scalar.dma_start(out=w3h[1][:],
                                in_=as_pkf(we3[0])[:, KH:DK, :])
            nc.scalar.dma_start(out=cb0[:], in_=cb[0])
            nc.sync.dma_start(out=w20[:], in_=as_pkf(we2[0]))

            def half_slices(tiles):
                def sl(k, csl):
                    return tiles[k // KH][:, k % KH, csl]
                return sl

            jobs.append((half_slices(w1h), half_slices(w3h),
                         lambda k, csl: w20[:, k, csl],
                         half_slices(xeh), cb0, as_pkf(oute[0]), C))

            for e in range(1, EPC):
                w1_t = wts.tile([128, DK, HID], bf16, name=f"w1_{e}")
                w3_t = wts.tile([128, DK, HID], bf16, name=f"w3_{e}")
                w2_t = wts.tile([128, HK, DIM], bf16, name=f"w2_{e}")
                x_t = actp.tile([128, DK, C], bf16, name=f"xe_{e}")
                cb_t = actp.tile([128, C], f32, name=f"cbt_{e}")
                nc.sync.dma_start(out=w1_t[:], in_=as_pkf(we1[e]))
                nc.scalar.dma_start(out=x_t[:], in_=as_pkf(xe[e]))
                nc.scalar.dma_start(out=w3_t[:], in_=as_pkf(we3[e]))
                nc.scalar.dma_start(out=cb_t[:], in_=cb[e])
                nc.sync.dma_start(out=w2_t[:], in_=as_pkf(we2[e]))

                def mk(t):
                    return lambda k, csl: t[:, k, csl]
                jobs.append((mk(w1_t), mk(w3_t), mk(w2_t), mk(x_t), cb_t,
                             as_pkf(oute[e]), C))

            w1_s = wts.tile([128, DK, HID], bf16, name="sw1")
            w3_s = wts.tile([128, DK, HID], bf16, name="sw3")
            w2_s = wts.tile([128, HK, DIM], bf16, name="sw2")
            x_s = actp.tile([128, DK, S], bf16, name="xst")
            nc.sync.dma_start(out=w1_s[:], in_=as_pkf(ws1[:]))
            nc.scalar.dma_start(out=x_s[:], in_=as_pkf(xs[:]))
            nc.scalar.dma_start(out=w3_s[:], in_=as_pkf(ws3[:]))
            nc.sync.dma_start(out=w2_s[:], in_=as_pkf(ws2[:]))

            def mk(t):
                return lambda k, csl: t[:, k, csl]
            jobs.append((mk(w1_s), mk(w3_s), mk(w2_s), mk(x_s), None,
                         as_pkf(outs[:]), S))

            for (w1f, w3f, w2f, xf_, cb_t, o_ap, ntok) in jobs:
                for (n0, n) in _chunks(ntok):
                    csl = slice(n0, n0 + n)
                    act_t = []
                    for hm in range(HK):
                        hsl = slice(hm * 128, (hm + 1) * 128)
                        p1 = ph.tile([128, 512], f32, tag="h1", name="p1")
                        p3 = ph.tile([128, 512], f32, tag="h3", name="p3")
                        for k in range(DK):
                            nc.tensor.matmul(p1[:, :n], w1f(k, hsl),
                                             xf_(k, csl),
                                             start=(k == 0),
                                             stop=(k == DK - 1))
                        for k in range(DK):
                            nc.tensor.matmul(p3[:, :n], w3f(k, hsl),
                                             xf_(k, csl),
                                             start=(k == 0),
                                             stop=(k == DK - 1))
                        sil = work.tile([128, 512], bf16, tag="sil",
                                        name="sil")
                        nc.scalar.activation(sil[:, :n], p1[:, :n],
                                             mybir.ActivationFunctionType.Silu)
                        a = work.tile([128, 512], bf16, tag=f"act{hm}",
                                      name=f"act{hm}")
                        if cb_t is not None:
                            h3s = work.tile([128, 512], bf16, tag="h3s",
                                            name="h3s")
                            nc.vector.tensor_tensor(h3s[:, :n], p3[:, :n],
                                                    cb_t[:, csl],
                                                    mybir.AluOpType.mult)
                            nc.vector.tensor_tensor(a[:, :n], h3s[:, :n],
                                                    sil[:, :n],
                                                    mybir.AluOpType.mult)
                        else:
                            nc.vector.tensor_tensor(a[:, :n], p3[:, :n],
                                                    sil[:, :n],
                                                    mybir.AluOpType.mult)
                        act_t.append(a)
                    stage = ostp.tile([128, DK, 512], fout, tag="stage",
                                      name="stage")
                    for dm in range(DK):
                        dsl = slice(dm * 128, (dm + 1) * 128)
                        pout = po.tile([128, 512], f32, tag="o", name="pout")
                        for k in range(HK):
                            nc.tensor.matmul(pout[:, :n], w2f(k, dsl),
                                             act_t[k][:, :n],
                                             start=(k == 0),
                                             stop=(k == HK - 1))
                        nc.vector.tensor_copy(out=stage[:, dm, :n],
                                              in_=pout[:, :n])
                    nc.sync.dma_start(out=o_ap[:, 0:DK // 2, csl],
                                      in_=stage[:, 0:DK // 2, :n])
                    nc.scalar.dma_start(out=o_ap[:, DK // 2:DK, csl],
                                        in_=stage[:, DK // 2:DK, :n])

    nc.compile()
    return nc


def _get_nc(C: int):
    if C not in _CACHE:
        _CACHE[C] = _build(C)
    return _CACHE[C]


LAST_RESULTS = None  # BassKernelResults from the most recent run (for test.py)


def kernel(x, gate_w, w1, w3, w2, sw1, sw3, sw2):
    global LAST_RESULTS
    from concourse.bass_utils import run_bass_kernel_spmd

    x = np.asarray(x)
    xf = np.ascontiguousarray(x.reshape(-1, DIM).astype(np.float32))
    gate_w = np.asarray(gate_w, dtype=np.float32)

    # ---- router on host (softmax -> top-4 -> renormalize) ----
    logits = xf @ gate_w.T                      # [T, E]
    m = logits.max(axis=1, keepdims=True)
    p = np.exp(logits - m)
    probs = p / p.sum(axis=1, keepdims=True)
    idx4 = np.argpartition(-probs, TOPK, axis=1)[:, :TOPK]     # [T, 4]
    w4 = np.take_along_axis(probs, idx4, axis=1)
    w4 = w4 / w4.sum(axis=1, keepdims=True)

    rows = np.repeat(np.arange(xf.shape[0]), TOPK)
    cols = idx4.ravel()
    vals = w4.ravel()

    tok_of = [rows[cols == e] for e in range(E)]
    cw_of = [vals[cols == e].astype(np.float32) for e in range(E)]
    counts = np.array([len(t) for t in tok_of])
    C = int(max(512, -(-counts.max() // 64) * 64))

    xf_bf = xf.astype(BF16)
    w1 = np.asarray(w1, dtype=np.float32)
    w3 = np.asarray(w3, dtype=np.float32)
    w2 = np.asarray(w2, dtype=np.float32)

    in_maps = []
    for c in range(NCORES):
        es = [c * EPC + j for j in range(EPC)]
        xe_np = np.zeros((EPC, DIM, C), dtype=BF16)
        cb_np = np.zeros((EPC, 128, C), dtype=np.float32)
        for j, e in enumerate(es):
            cnt = counts[e]
            xe_np[j, :, :cnt] = xf_bf[tok_of[e]].T
            cb_np[j, :, :cnt] = cw_of[e][None, :]
        im = {
            "xe": xe_np,
            "cb": cb_np,
            "we1": np.ascontiguousarray(
                w1[es].transpose(0, 2, 1)).astype(BF16),
            "we3": np.ascontiguousarray(
                w3[es].transpose(0, 2, 1)).astype(BF16),
            "we2": np.ascontiguousarray(
                w2[es].transpose(0, 2, 1)).astype(BF16),
            "xs": np.ascontiguousarray(xf_bf[c * S:(c + 1) * S].T),
            "ws1": np.ascontiguousarray(np.asarray(sw1, np.float32).T).astype(BF16),
            "ws3": np.ascontiguousarray(np.asarray(sw3, np.float32).T).astype(BF16),
            "ws2": np.ascontiguousarray(np.asarray(sw2, np.float32).T).astype(BF16),
        }
        in_maps.append(im)

    nc = _get_nc(C)
    trace = os.environ.get("KERNEL_TRACE", "0") == "1"
    res = run_bass_kernel_spmd(nc, in_maps, core_ids=list(range(NCORES)),
                               trace=trace)
    LAST_RESULTS = res

    out = np.zeros((T, DIM), dtype=np.float32)
    for c in range(NCORES):
        r = res.results[c]
        for j in range(EPC):
            e = c * EPC + j
            cnt = counts[e]
            out[tok_of[e]] += r["oute"][j, :, :cnt].T.astype(np.float32)
        out[c * S:(c + 1) * S] += r["outs"].T.astype(np.float32)
    return out.reshape(x.shape).astype(np.float32)


# revision 6
# speedup vs baseline: 1.2963x; 1.2963x over previous
"""MoE layer (16 experts, top-4, silu-gated FFN + shared expert) on 8 trn2 cores.

Strategy (expert-parallel, host-side dispatch):
  - Host computes the router (softmax + top-4 + renormalize) in numpy —
    0.2% of total FLOPs — and gathers each expert's tokens into a padded
    [capacity] batch (classic MoE dispatch, done host-side instead of
    device all-to-all).
  - Each of the 8 cores holds 2 experts (weights resident in SBUF, bf16)
    and runs the dense silu-gated FFN over its experts' gathered tokens,
    scaling activations by the combine weights before the down-projection
    so partial outputs can be scatter-added on the host.
  - The shared expert is data-parallel: core i handles tokens
    [i*256, (i+1)*256).
  - All activations/weights are bf16 (PE: 1 cycle/row vs 2 for fp32),
    accumulation in fp32 PSUM.

Device layout: activations kept transposed (feature on partitions, tokens
on the free dim) so both matmuls feed the PE without any on-device
transpose; combine weights arrive pre-broadcast as [128, C] rows. All DRAM
tensors are partition-major ([128, k*f]: SBUF partition is the leading
axis) so every DMA moves multi-KB contiguous segments per partition — with
the natural [(k p), f] layout the 1KB-row packets capped the single HWDGE
queue at ~220 GB/s. Outputs are chunk-major for the same reason. Expert
0's first-needed tensors load in k-halves so the PE can start early. Token
chunks are equal halves (288+288 for C=576) so no chunk is so short that
LDWEIGHTS dominates.
"""

import os
import numpy as np
import ml_dtypes

DIM = 1024
HID = 512
E = 16
TOPK = 4
NCORES = 8
EPC = E // NCORES  # experts per core
T = 2048
S = T // NCORES  # shared-expert tokens per core

BF16 = ml_dtypes.bfloat16
OUT_BF16 = os.environ.get("KERNEL_OUT_F32", "0") != "1"

DK = DIM // 128   # 8 contraction tiles for the up-projections
HK = HID // 128   # 4 contraction tiles for the down-projection
KH = DK // 2

_CACHE = {}


def _chunks(total):
    if total <= 512:
        return [(0, total)]
    nch = -(-total // 512)
    base = -(-total // (nch * 16)) * 16
    out, n0 = [], 0
    while n0 < total:
        n = min(base, total - n0)
        out.append((n0, n))
        n0 += n
    return out


def _build(C: int):
    """Build + schedule the SPMD Tile kernel for per-expert capacity C."""
    import concourse.tile as tile
    import concourse.mybir as mybir
    from concourse import bacc

    f32 = mybir.dt.float32
    bf16 = mybir.dt.bfloat16
    fout = bf16 if OUT_BF16 else f32

    nc = bacc.Bacc("TRN2", target_bir_lowering=False, debug=False,
                   num_devices=NCORES)

    # all DRAM tensors partition-major: [128, k*f]
    xe = nc.dram_tensor("xe", [EPC, 128, DK * C], bf16, kind="ExternalInput")
    cb = nc.dram_tensor("cb", [EPC, 128, C], f32, kind="ExternalInput")
    we1 = nc.dram_tensor("we1", [EPC, 128, DK * HID], bf16, kind="ExternalInput")
    we3 = nc.dram_tensor("we3", [EPC, 128, DK * HID], bf16, kind="ExternalInput")
    we2 = nc.dram_tensor("we2", [EPC, 128, HK * DIM], bf16, kind="ExternalInput")
    xs = nc.dram_tensor("xs", [128, DK * S], bf16, kind="ExternalInput")
    ws1 = nc.dram_tensor("ws1", [128, DK * HID], bf16, kind="ExternalInput")
    ws3 = nc.dram_tensor("ws3", [128, DK * HID], bf16, kind="ExternalInput")
    ws2 = nc.dram_tensor("ws2", [128, HK * DIM], bf16, kind="ExternalInput")
    # outputs chunk-major: chunk (n0, n) occupies cols [DK*n0, DK*(n0+n))
    oute = nc.dram_tensor("oute", [EPC, 128, DK * C], fout,
                          kind="ExternalOutput")
    outs = nc.dram_tensor("outs", [128, DK * S], fout, kind="ExternalOutput")

    def k3(ap, k):
        return ap.rearrange("p (k f) -> p k f", k=k)

    with tile.TileContext(nc) as tc:
        with (
            tc.tile_pool(name="wts", bufs=1) as wts,
            tc.tile_pool(name="acts", bufs=1) as actp,
            tc.tile_pool(name="work", bufs=2) as work,
            tc.tile_pool(name="ost", bufs=2) as ostp,
            tc.tile_pool(name="ph", bufs=2, space="PSUM") as ph,
            tc.tile_pool(name="po", bufs=2, space="PSUM") as po,
        ):
            jobs = []
            # expert 0: first-needed tensors in k-halves for a fast start
            w1h = [wts.tile([128, KH, HID], bf16, name=f"w1_0{h}")
                   for h in range(2)]
            w3h = [wts.tile([128, KH, HID], bf16, name=f"w3_0{h}")
                   for h in range(2)]
            xeh = [actp.tile([128, KH, C], bf16, name=f"xe_0{h}")
                   for h in range(2)]
            cb0 = actp.tile([128, C], f32, name="cbt_0")
            w20 = wts.tile([128, HK, DIM], bf16, name="w2_0")
            nc.sync.dma_start(out=w1h[0][:], in_=k3(we1[0], DK)[:, 0:KH, :])
            nc.sync.dma_start(out=xeh[0][:], in_=k3(xe[0], DK)[:, 0:KH, :])
            nc.sync.dma_start(out=w1h[1][:], in_=k3(we1[0], DK)[:, KH:DK, :])
            nc.sync.dma_start(out=xeh[1][:], in_=k3(xe[0], DK)[:, KH:DK, :])
            nc.sync.dma_start(out=w3h[0][:], in_=k3(we3[0], DK)[:, 0:KH, :])
            nc.sync.dma_start(out=w3h[1][:], in_=k3(we3[0], DK)[:, KH:DK, :])
            nc.sync.dma_start(out=cb0[:], in_=cb[0])
            nc.sync.dma_start(out=w20[:], in_=k3(we2[0], HK))

            def half_slices(tiles):
                def sl(k, csl):
                    return tiles[k // KH][:, k % KH, csl]
                return sl

            jobs.append((half_slices(w1h), half_slices(w3h),
                         lambda k, csl: w20[:, k, csl],
                         half_slices(xeh), cb0, oute[0], C))

            for e in range(1, EPC):
                w1_t = wts.tile([128, DK, HID], bf16, name=f"w1_{e}")
                w3_t = wts.tile([128, DK, HID], bf16, name=f"w3_{e}")
                w2_t = wts.tile([128, HK, DIM], bf16, name=f"w2_{e}")
                x_t = actp.tile([128, DK, C], bf16, name=f"xe_{e}")
                cb_t = actp.tile([128, C], f32, name=f"cbt_{e}")
                nc.sync.dma_start(out=w1_t[:], in_=k3(we1[e], DK))
                nc.sync.dma_start(out=x_t[:], in_=k3(xe[e], DK))
                nc.sync.dma_start(out=w3_t[:], in_=k3(we3[e], DK))
                nc.sync.dma_start(out=cb_t[:], in_=cb[e])
                nc.sync.dma_start(out=w2_t[:], in_=k3(we2[e], HK))

                def mk(t):
                    return lambda k, csl: t[:, k, csl]
                jobs.append((mk(w1_t), mk(w3_t), mk(w2_t), mk(x_t), cb_t,
                             oute[e], C))

            w1_s = wts.tile([128, DK, HID], bf16, name="sw1")
            w3_s = wts.tile([128, DK, HID], bf16, name="sw3")
            w2_s = wts.tile([128, HK, DIM], bf16, name="sw2")
            x_s = actp.tile([128, DK, S], bf16, name="xst")
            nc.sync.dma_start(out=w1_s[:], in_=k3(ws1[:], DK))
            nc.sync.dma_start(out=x_s[:], in_=k3(xs[:], DK))
            nc.sync.dma_start(out=w3_s[:], in_=k3(ws3[:], DK))
            nc.sync.dma_start(out=w2_s[:], in_=k3(ws2[:], HK))

            def mk(t):
                return lambda k, csl: t[:, k, csl]
            jobs.append((mk(w1_s), mk(w3_s), mk(w2_s), mk(x_s), None,
                         outs[:], S))

            for (w1f, w3f, w2f, xf_, cb_t, o_ap, ntok) in jobs:
                for (n0, n) in _chunks(ntok):
                    csl = slice(n0, n0 + n)
                    act_t = []
                    for hm in range(HK):
                        hsl = slice(hm * 128, (hm + 1) * 128)
                        p1 = ph.tile([128, 512], f32, tag="h1", name="p1")
                        p3 = ph.tile([128, 512], f32, tag="h3", name="p3")
                        for k in range(DK):
                            nc.tensor.matmul(p1[:, :n], w1f(k, hsl),
                                             xf_(k, csl),
                                             start=(k == 0),
                                             stop=(k == DK - 1))
                        for k in range(DK):
                            nc.tensor.matmul(p3[:, :n], w3f(k, hsl),
                                             xf_(k, csl),
                                             start=(k == 0),
                                             stop=(k == DK - 1))
                        sil = work.tile([128, 512], bf16, tag="sil",
                                        name="sil")
                        nc.scalar.activation(sil[:, :n], p1[:, :n],
                                             mybir.ActivationFunctionType.Silu)
                        a = work.tile([128, 512], bf16, tag=f"act{hm}",
                                      name=f"act{hm}")
                        if cb_t is not None:
                            h3s = work.tile([128, 512], bf16, tag="h3s",
                                            name="h3s")
                            nc.vector.tensor_tensor(h3s[:, :n], p3[:, :n],
                                                    cb_t[:, csl],
                                                    mybir.AluOpType.mult)
                            nc.vector.tensor_tensor(a[:, :n], h3s[:, :n],
                                                    sil[:, :n],
                                                    mybir.AluOpType.mult)
                        else:
                            nc.vector.tensor_tensor(a[:, :n], p3[:, :n],
                                                    sil[:, :n],
                                                    mybir.AluOpType.mult)
                        act_t.append(a)
                    stage = ostp.tile([128, DK, 512], fout, tag="stage",
                                      name="stage")
                    for dm in range(DK):
                        dsl = slice(dm * 128, (dm + 1) * 128)
                        pout = po.tile([128, 512], f32, tag="o", name="pout")
                        for k in range(HK):
                            nc.tensor.matmul(pout[:, :n], w2f(k, dsl),
                                             act_t[k][:, :n],
                                             start=(k == 0),
                                             stop=(k == HK - 1))
                        nc.vector.tensor_copy(out=stage[:, dm, :n],
                                              in_=pout[:, :n])
                    o_chunk = o_ap[:, DK * n0:DK * (n0 + n)].rearrange(
                        "p (k t) -> p k t", k=DK)
                    nc.sync.dma_start(out=o_chunk, in_=stage[:, :, :n])

    nc.compile()
    return nc


def _get_nc(C: int):
    if C not in _CACHE:
        _CACHE[C] = _build(C)
    return _CACHE[C]


def _pmajor(a, nk):
    """[(k p), f] -> [128, k, f] partition-major view for DMA-friendly rows."""
    kp, f = a.shape
    return np.ascontiguousarray(
        a.reshape(nk, 128, f).transpose(1, 0, 2))


LAST_RESULTS = None  # BassKernelResults from the most recent run (for test.py)


def kernel(x, gate_w, w1, w3, w2, sw1, sw3, sw2):
    global LAST_RESULTS
    from concourse.bass_utils import run_bass_kernel_spmd

    x = np.asarray(x)
    xf = np.ascontiguousarray(x.reshape(-1, DIM).astype(np.float32))
    gate_w = np.asarray(gate_w, dtype=np.float32)

    # ---- router on host (softmax -> top-4 -> renormalize) ----
    logits = xf @ gate_w.T                      # [T, E]
    m = logits.max(axis=1, keepdims=True)
    p = np.exp(logits - m)
    probs = p / p.sum(axis=1, keepdims=True)
    idx4 = np.argpartition(-probs, TOPK, axis=1)[:, :TOPK]     # [T, 4]
    w4 = np.take_along_axis(probs, idx4, axis=1)
    w4 = w4 / w4.sum(axis=1, keepdims=True)

    rows = np.repeat(np.arange(xf.shape[0]), TOPK)
    cols = idx4.ravel()
    vals = w4.ravel()

    tok_of = [rows[cols == e] for e in range(E)]
    cw_of = [vals[cols == e].astype(np.float32) for e in range(E)]
    counts = np.array([len(t) for t in tok_of])
    C = int(max(512, -(-counts.max() // 64) * 64))

    xf_bf = xf.astype(BF16)
    w1 = np.asarray(w1, dtype=np.float32)
    w3 = np.asarray(w3, dtype=np.float32)
    w2 = np.asarray(w2, dtype=np.float32)
    sw1T = _pmajor(np.asarray(sw1, np.float32).T.astype(BF16), DK)
    sw3T = _pmajor(np.asarray(sw3, np.float32).T.astype(BF16), DK)
    sw2T = _pmajor(np.asarray(sw2, np.float32).T.astype(BF16), HK)

    in_maps = []
    for c in range(NCORES):
        es = [c * EPC + j for j in range(EPC)]
        xe_np = np.zeros((EPC, 128, DK, C), dtype=BF16)
        cb_np = np.zeros((EPC, 128, C), dtype=np.float32)
        for j, e in enumerate(es):
            cnt = counts[e]
            g = xf_bf[tok_of[e]].T                 # [(k p), cnt]
            xe_np[j, :, :, :cnt] = g.reshape(DK, 128, cnt).transpose(1, 0, 2)
            cb_np[j, :, :cnt] = cw_of[e][None, :]
        im = {
            "xe": xe_np.reshape(EPC, 128, DK * C),
            "cb": cb_np,
            "we1": np.stack([_pmajor(w1[e].T.astype(BF16), DK) for e in es]
                            ).reshape(EPC, 128, DK * HID),
            "we3": np.stack([_pmajor(w3[e].T.astype(BF16), DK) for e in es]
                            ).reshape(EPC, 128, DK * HID),
            "we2": np.stack([_pmajor(w2[e].T.astype(BF16), HK) for e in es]
                            ).reshape(EPC, 128, HK * DIM),
            "xs": _pmajor(xf_bf[c * S:(c + 1) * S].T, DK
                          ).reshape(128, DK * S),
            "ws1": sw1T.reshape(128, DK * HID),
            "ws3": sw3T.reshape(128, DK * HID),
            "ws2": sw2T.reshape(128, HK * DIM),
        }
        in_maps.append(im)

    nc = _get_nc(C)
    trace = os.environ.get("KERNEL_TRACE", "0") == "1"
    res = run_bass_kernel_spmd(nc, in_maps, core_ids=list(range(NCORES)),
                               trace=trace)
    LAST_RESULTS = res

    def decode(arr, ntok):
        """chunk-major [128, DK*ntok] -> [ntok, DIM] (token-major)."""
        outT = np.empty((DIM, ntok), dtype=np.float32)
        for (n0, n) in _chunks(ntok):
            blk = arr[:, DK * n0:DK * (n0 + n)].astype(np.float32)
            outT[:, n0:n0 + n] = blk.reshape(128, DK, n).transpose(
                1, 0, 2).reshape(DIM, n)
        return outT.T

    out = np.zeros((T, DIM), dtype=np.float32)
    for c in range(NCORES):
        r = res.results[c]
        for j in range(EPC):
            e = c * EPC + j
            cnt = counts[e]
            out[tok_of[e]] += decode(r["oute"][j], C)[:cnt]
        out[c * S:(c + 1) * S] += decode(r["outs"], S)
    return out.reshape(x.shape).astype(np.float32)
